# revision 6
# baseline (speedup 1.0000x reference)
"""Multi-head SwiGLU feed-forward (nn_MultiHeadFeedForward) Trainium2 kernel.

Math (per head h of 16, head_dim d=128, ffn f=512):
    g = x_h @ gate_w[h]      # [T,128]@[128,512]
    u = x_h @ up_w[h]
    out_h = (silu(g)*u) @ down_w[h]   # [T,512]@[512,128]

Sharding: 2 heads per core across 8 cores (no cross-core communication).
On-chip layout is feature-major ("transposed"): activations live as
[feature, token] tiles so every matmul contracts along the partition dim
without any on-chip transposes.  The host pre-transposes x into
xT[h, d, t] per core and un-transposes the output.
"""

import os
import sys

import numpy as np

for _p in ("/opt/trn_rl_repo",):
    if _p not in sys.path and os.path.isdir(_p):
        sys.path.insert(0, _p)

import concourse.bass as bass
import concourse.mybir as mybir
from concourse import bacc
import concourse.tile as tile
from concourse.bass_utils import run_bass_kernel_spmd

B, S, EMB = 4, 4096, 2048
HEADS, HD, FFN = 16, 128, 512
T = B * S                      # 16384 tokens
N_CORES = 8
HPC = HEADS // N_CORES         # heads per core = 2
TOK = 256                      # tokens per on-chip tile
NT = T // TOK                  # token tiles per head
NCH = FFN // HD                # ffn chunks of 128 = 4
SLAB = 4096                    # output slab tokens (drained by 2 half DMAs)

F32 = mybir.dt.float32
BF16 = mybir.dt.bfloat16
AF = mybir.ActivationFunctionType


def _build_nc():
    nc = bacc.Bacc("TRN2", target_bir_lowering=False)

    xT = nc.dram_tensor("xT", [HPC, HD, T], BF16, kind="ExternalInput")
    gw = nc.dram_tensor("gw", [HPC, HD, FFN], BF16, kind="ExternalInput")
    uw = nc.dram_tensor("uw", [HPC, HD, FFN], BF16, kind="ExternalInput")
    dw = nc.dram_tensor("dw", [HPC, FFN, HD], BF16, kind="ExternalInput")
    outT = nc.dram_tensor("outT", [HPC, HD, T], BF16, kind="ExternalOutput")

    # Output accumulates in SBUF slabs of SLAB tokens, drained by two large
    # read-only DMAs each (one per copy-engine half) so every DMA needs at
    # most ONE semaphore wait (walrus DIRECT2D DMAs only support one).
    TPS = SLAB // TOK  # tiles per slab

    with tile.TileContext(nc) as tc:
        with (
            tc.tile_pool(name="wpool", bufs=1) as wpool,
            tc.tile_pool(name="gpool", bufs=2, space="PSUM") as gpool,
            tc.tile_pool(name="upool", bufs=2, space="PSUM") as upool,
            tc.tile_pool(name="sgpool", bufs=4) as sgpool,
            tc.tile_pool(name="hpool", bufs=4) as hpool,
            tc.tile_pool(name="slabs", bufs=3) as slabs,
        ):
            # weights + the entire x shard resident in SBUF for the kernel
            gw_s = wpool.tile([HD, HPC, FFN], BF16)
            uw_s = wpool.tile([HD, HPC, FFN], BF16)
            dw_s = wpool.tile([HD, HPC, NCH, HD], BF16)
            xs_full = wpool.tile([HD, HPC, T], BF16)
            # Every [128, N] DMA is ~128 serial descriptors (~74ns each) on a
            # single queue; split the latency-critical first-tile loads across
            # partition halves (2 queues) so compute can start ~2x sooner, and
            # issue them before the bulk x stream.
            XC = 512

            def split_dma(out, in_, ways=2):
                step = HD // ways
                for s in range(ways):
                    p = slice(s * step, (s + 1) * step)
                    nc.sync.dma_start(out=out[p], in_=in_[p])

            for h in range(HPC):
                split_dma(gw_s[:, h, :], gw[h])
                split_dma(uw_s[:, h, :], uw[h])
                split_dma(
                    dw_s[:, h, :, :],
                    dw[h].rearrange("(c p) d -> p c d", p=HD),
                )
            # first x chunk of head 0 (feeds tile 0) ahead of everything else
            split_dma(xs_full[:, 0, 0:XC], xT[0, :, 0:XC], ways=4)
            for h in range(HPC):
                for xc in range(T // XC):
                    if h == 0 and xc == 0:
                        continue
                    c0 = xc * XC
                    nc.sync.dma_start(
                        out=xs_full[:, h, c0 : c0 + XC],
                        in_=xT[h, :, c0 : c0 + XC],
                    )

            # Software pipeline with a 2-tile lag on the down-proj: tile k's
            # down-proj + slab copy are emitted in iteration k+2, when every
            # dependency (hh(k), silu(k+1)'s read of the overlay bank) has
            # already retired, so neither PE nor ACT/DVE ever queue a stalled
            # instruction ahead of ready work.  The down-proj PSUM output is
            # overlaid into the gate-psum banks of tile k+1 (consumed by
            # silu(k+1), recycled by gate(k+3)), keeping total PSUM usage at
            # 8 banks with everything double-buffered.
            slab = None
            pend = []  # [(hh, slab, h, t, o_target), ...] oldest first

            def emit_down_pair(p):
                # down-proj for a PAIR of tiles: 4 matmuls of N=2*TOK reading
                # the pair's joint hh, accumulating into one full PSUM bank;
                # then one [128, 2*TOK] copy to the slab.  Copy engine
                # alternates by slab quarter so each quarter-drain DMA waits
                # on a single engine.
                phh, pslab, ph, pt, ops = p  # pt = SECOND tile of the pair
                for c in range(NCH):
                    nc.tensor.matmul(
                        ops,
                        lhsT=dw_s[:, ph, c, :],
                        rhs=phh[:, c, :],
                        start=(c == 0),
                        stop=(c == NCH - 1),
                    )
                pts = pt % TPS
                dst = pslab[:, (pts - 1) * TOK : (pts + 1) * TOK]
                QT = TPS // 4  # tiles per drained quarter
                q = pts // QT
                # 3:1 ACT:DVE drain split — DVE (mul, 1.042ns/elem) is the
                # most-loaded engine; ACT (silu, 0.833ns/elem) has slack.
                # Keep each quarter single-engine: its drain DMA then needs
                # only one semaphore wait.
                if q % 4 != 3:
                    nc.scalar.copy(dst, ops)
                else:
                    nc.vector.tensor_copy(dst, ops)
                if pts % QT == QT - 1:
                    pt0 = pt * TOK
                    for s in range(2):  # partition-split across 2 queues
                        p = slice(s * 64, (s + 1) * 64)
                        nc.sync.dma_start(
                            out=outT[ph, p, pt0 + TOK - QT * TOK : pt0 + TOK],
                            in_=pslab[p, q * QT * TOK : (q + 1) * QT * TOK],
                        )

            tiles = [(h, t) for h in range(HPC) for t in range(NT)]
            K = len(tiles)

            def emit_gate(k):
                h, t = tiles[k]
                xs = xs_full[:, h, t * TOK : (t + 1) * TOK]
                gps = gpool.tile([HD, NCH * TOK], F32, name=f"gps_{k}", tag="g")
                for c in range(NCH):
                    nc.tensor.matmul(
                        gps[:, c * TOK : (c + 1) * TOK],
                        lhsT=gw_s[:, h, c * HD : (c + 1) * HD],
                        rhs=xs,
                        start=True,
                        stop=True,
                    )
                sg = sgpool.tile([HD, NCH * TOK], BF16, name=f"sg_{k}", tag="sg")
                nc.scalar.activation(sg[:], gps[:], AF.Silu)
                return gps, sg

            # prologue: gate+silu for tile 0
            gate_next = emit_gate(0)
            hh_pair = None
            for k in range(K):
                h, t = tiles[k]
                if t % TPS == 0:
                    slab = slabs.tile([HD, SLAB], BF16, name=f"slab_{k}", tag="slab")

                # pending pair's down-proj + slab copy (2-tile lag: all deps
                # retired by now)
                if k % 2 == 1 and pend:
                    emit_down_pair(pend.pop(0))

                gps, sg = gate_next
                # patch the previous pair's overlay target to THIS (even)
                # tile's gate bank 0: consumed by silu(k), recycled only by
                # gate(k+2) a full period after the pair's copy
                if k % 2 == 0 and pend:
                    pend[-1] = pend[-1][:4] + (gps[:, : 2 * TOK],)

                ups = upool.tile([HD, NCH * TOK], F32, name=f"ups_{k}", tag="u")
                xs = xs_full[:, h, t * TOK : (t + 1) * TOK]
                for c in range(NCH):
                    nc.tensor.matmul(
                        ups[:, c * TOK : (c + 1) * TOK],
                        lhsT=uw_s[:, h, c * HD : (c + 1) * HD],
                        rhs=xs,
                        start=True,
                        stop=True,
                    )
                # next tile's gate+silu ahead of this tile's mul: PE runs it
                # during the mul; silu(k+1) overlaps mul(k) on ACT
                if k + 1 < K:
                    gate_next = emit_gate(k + 1)
                if k % 2 == 0:
                    hh_pair = hpool.tile(
                        [HD, NCH, 2 * TOK], BF16, name=f"hh_{k}", tag="hh"
                    )
                half = hh_pair[:, :, (k % 2) * TOK : (k % 2 + 1) * TOK]
                nc.vector.tensor_mul(
                    half,
                    sg[:].rearrange("p (c n) -> p c n", c=NCH),
                    ups[:].rearrange("p (c n) -> p c n", c=NCH),
                )

                if k % 2 == 1:
                    # pair (k-1, k) complete; its down-proj (emitted next
                    # iteration) accumulates into THIS tile's gate bank 0,
                    # already consumed by silu(k)
                    pend.append((hh_pair, slab, h, t, gps[:, : 2 * TOK]))
            # epilogue
            for p in pend:
                emit_down_pair(p)
    nc.compile()
    return nc


def _shard_inputs(inputs):
    import ml_dtypes

    bf16 = ml_dtypes.bfloat16
    x = np.asarray(inputs["x"], dtype=np.float32)
    gw = np.asarray(inputs["gate_w"], dtype=np.float32).astype(bf16)
    uw = np.asarray(inputs["up_w"], dtype=np.float32).astype(bf16)
    dw = np.asarray(inputs["down_w"], dtype=np.float32).astype(bf16)

    xh = x.reshape(T, HEADS, HD)
    xt = np.ascontiguousarray(xh.transpose(1, 2, 0)).astype(bf16)  # [16, 128, T]

    in_maps = []
    for c in range(N_CORES):
        hs = slice(HPC * c, HPC * (c + 1))
        in_maps.append(
            {
                "xT": xt[hs],
                "gw": gw[hs],
                "uw": uw[hs],
                "dw": dw[hs],
            }
        )
    return in_maps


def run(inputs, trace=False, **spmd_kwargs):
    nc = _build_nc()
    in_maps = _shard_inputs(inputs)
    res = run_bass_kernel_spmd(
        nc, in_maps, core_ids=list(range(N_CORES)), trace=trace, **spmd_kwargs
    )
    outT = np.empty((HEADS, HD, T), dtype=np.float32)
    for c in range(N_CORES):
        outT[HPC * c : HPC * (c + 1)] = np.asarray(
            res.results[c]["outT"], dtype=np.float32
        )
    out = np.ascontiguousarray(outT.transpose(2, 0, 1)).reshape(B, S, EMB)
    return out, res


def kernel(**inputs):
    out, _ = run(inputs)
    return out



# revision 8
# speedup vs baseline: 1.0366x; 1.0366x over previous
"""Multi-head SwiGLU feed-forward (nn_MultiHeadFeedForward) Trainium2 kernel.

Math (per head h of 16, head_dim d=128, ffn f=512):
    g = x_h @ gate_w[h]      # [T,128]@[128,512]
    u = x_h @ up_w[h]
    out_h = (silu(g)*u) @ down_w[h]   # [T,512]@[512,128]

Sharding: 2 heads per core across 8 cores (no cross-core communication).
On-chip layout is feature-major ("transposed"): activations live as
[feature, token] tiles so every matmul contracts along the partition dim
without any on-chip transposes.  The host pre-transposes x into
xT[h, d, t] per core and un-transposes the output.

Steady-state budget per 2-tile pair (512 tokens), from the hw cost model:
  PE   8 N=256 gate/up MMs + 4 N=512 down MMs  = 2608 ns   <- pacer
  ACT  2 silu [128,1024] + 0.5 drain copies    = 2513 ns
  DVE  2 mul  [128,1024] + 0.5 drain copies    = 2583 ns
The down-proj runs with a 4-tile lag so its PSUM-overlay and hh deps are
long retired and its matmuls never wait on a just-finished DVE mul.
"""

import os
import sys

import numpy as np

for _p in ("/opt/trn_rl_repo",):
    if _p not in sys.path and os.path.isdir(_p):
        sys.path.insert(0, _p)

import concourse.bass as bass
import concourse.mybir as mybir
from concourse import bacc
import concourse.tile as tile
from concourse.bass_utils import run_bass_kernel_spmd

B, S, EMB = 4, 4096, 2048
HEADS, HD, FFN = 16, 128, 512
T = B * S                      # 16384 tokens
N_CORES = 8
HPC = HEADS // N_CORES         # heads per core = 2
TOK = 256                      # tokens per on-chip tile
NT = T // TOK                  # token tiles per head
NCH = FFN // HD                # ffn chunks of 128 = 4

F32 = mybir.dt.float32
BF16 = mybir.dt.bfloat16
AF = mybir.ActivationFunctionType


def _build_nc():
    nc = bacc.Bacc("TRN2", target_bir_lowering=False)

    xT = nc.dram_tensor("xT", [HPC, HD, T], BF16, kind="ExternalInput")
    gw = nc.dram_tensor("gw", [HPC, HD, FFN], BF16, kind="ExternalInput")
    uw = nc.dram_tensor("uw", [HPC, HD, FFN], BF16, kind="ExternalInput")
    dw = nc.dram_tensor("dw", [HPC, FFN, HD], BF16, kind="ExternalInput")
    outT = nc.dram_tensor("outT", [HPC, HD, T], BF16, kind="ExternalOutput")

    with tile.TileContext(nc) as tc:
        with (
            tc.tile_pool(name="wpool", bufs=1) as wpool,
            tc.tile_pool(name="gpool", bufs=2, space="PSUM") as gpool,
            tc.tile_pool(name="upool", bufs=2, space="PSUM") as upool,
            tc.tile_pool(name="sgpool", bufs=4) as sgpool,
            tc.tile_pool(name="hpool", bufs=4) as hpool,
            tc.tile_pool(name="opool", bufs=8) as opool,
        ):
            # weights + the entire x shard resident in SBUF for the kernel
            gw_s = wpool.tile([HD, HPC, FFN], BF16)
            uw_s = wpool.tile([HD, HPC, FFN], BF16)
            dw_s = wpool.tile([HD, HPC, NCH, HD], BF16)
            xs_full = wpool.tile([HD, HPC, T], BF16)
            # DMA issue costs ~650ns per dma_start on the sync sequencer, so
            # order matters more than splitting: the first gate matmul needs
            # gw[0] + the first x tokens, so those two calls go first.
            XC = 1024
            nc.sync.dma_start(out=gw_s[:, 0, :], in_=gw[0])
            # small first chunk so tile 0 can start ~4us sooner
            nc.sync.dma_start(out=xs_full[:, 0, 0:512], in_=xT[0, :, 0:512])
            nc.sync.dma_start(out=uw_s[:, 0, :], in_=uw[0])
            nc.sync.dma_start(
                out=dw_s[:, 0, :, :],
                in_=dw[0].rearrange("(c p) d -> p c d", p=HD),
            )
            nc.sync.dma_start(out=xs_full[:, 0, 512:XC], in_=xT[0, :, 512:XC])
            for h in range(1, HPC):
                nc.sync.dma_start(out=gw_s[:, h, :], in_=gw[h])
                nc.sync.dma_start(out=uw_s[:, h, :], in_=uw[h])
                nc.sync.dma_start(
                    out=dw_s[:, h, :, :],
                    in_=dw[h].rearrange("(c p) d -> p c d", p=HD),
                )
            for h in range(HPC):
                for xc in range(T // XC):
                    if h == 0 and xc == 0:
                        continue
                    c0 = xc * XC
                    nc.sync.dma_start(
                        out=xs_full[:, h, c0 : c0 + XC],
                        in_=xT[h, :, c0 : c0 + XC],
                    )

            tiles = [(h, t) for h in range(HPC) for t in range(NT)]
            K = len(tiles)
            pend = []       # [(hh, h, t), ...] oldest first
            gps_hist = {}   # tile index -> gate psum tile (last 2 kept)
            n_pairs = [0]

            def emit_down_pair(p, ops):
                # down-proj for a PAIR of tiles: 4 matmuls of N=2*TOK reading
                # the pair's joint hh, accumulating into the overlay PSUM
                # region `ops` (bank 0 of a retired gate-psum tile); then one
                # [128, 512] copy into a small out buffer, alternating engine
                # per pair (keeps ACT/DVE evenly loaded and each out-DMA
                # waiting on a single engine), then one DMA per pair.
                phh, ph, pt = p
                for c in range(NCH):
                    nc.tensor.matmul(
                        ops,
                        lhsT=dw_s[:, ph, c, :],
                        rhs=phh[:, c, :],
                        start=(c == 0),
                        stop=(c == NCH - 1),
                    )
                ob = opool.tile([HD, 2 * TOK], BF16, name=f"ob_{ph}_{pt}", tag="ob")
                if n_pairs[0] % 2 == 0:
                    nc.scalar.copy(ob[:], ops)
                else:
                    nc.vector.tensor_copy(ob[:], ops)
                n_pairs[0] += 1
                pt0 = (pt - 1) * TOK
                if n_pairs[0] > 62:
                    # tail: partition-split so the final DMA isn't a 9.5us
                    # single-queue descriptor chain
                    for s in range(2):
                        pp = slice(s * 64, (s + 1) * 64)
                        nc.sync.dma_start(
                            out=outT[ph, pp, pt0 : pt0 + 2 * TOK],
                            in_=ob[pp, :],
                        )
                else:
                    nc.sync.dma_start(
                        out=outT[ph, :, pt0 : pt0 + 2 * TOK], in_=ob[:, :]
                    )

            def emit_gate(k):
                h, t = tiles[k]
                xs = xs_full[:, h, t * TOK : (t + 1) * TOK]
                gps = gpool.tile([HD, NCH * TOK], F32, name=f"gps_{k}", tag="g")
                for c in range(NCH):
                    nc.tensor.matmul(
                        gps[:, c * TOK : (c + 1) * TOK],
                        lhsT=gw_s[:, h, c * HD : (c + 1) * HD],
                        rhs=xs,
                        start=True,
                        stop=True,
                    )
                sg = sgpool.tile([HD, NCH * TOK], BF16, name=f"sg_{k}", tag="sg")
                nc.scalar.activation(sg[:], gps[:], AF.Silu)
                gps_hist[k] = gps
                gps_hist.pop(k - 3, None)
                return gps, sg

            # prologue: gate+silu for tile 0
            gate_next = emit_gate(0)
            hh_pair = None
            for k in range(K):
                h, t = tiles[k]
                # pop a pending pair with a 4-tile lag: overlay into the gate
                # psum of tile k-1 (its silu retired a full iteration ago; its
                # pool slot is reallocated by gate(k+1) AFTER this emission,
                # so the pool inserts the copy->gate(k+1) dependency)
                if k % 2 == 1 and len(pend) > 1:
                    emit_down_pair(pend.pop(0), gps_hist[k - 1][:, : 2 * TOK])

                gps, sg = gate_next
                ups = upool.tile([HD, NCH * TOK], F32, name=f"ups_{k}", tag="u")
                xs = xs_full[:, h, t * TOK : (t + 1) * TOK]
                for c in range(NCH):
                    nc.tensor.matmul(
                        ups[:, c * TOK : (c + 1) * TOK],
                        lhsT=uw_s[:, h, c * HD : (c + 1) * HD],
                        rhs=xs,
                        start=True,
                        stop=True,
                    )
                # next tile's gate+silu ahead of this tile's mul: PE runs it
                # during the mul; silu(k+1) overlaps mul(k) on ACT
                if k + 1 < K:
                    gate_next = emit_gate(k + 1)
                if k % 2 == 0:
                    hh_pair = hpool.tile(
                        [HD, NCH, 2 * TOK], BF16, name=f"hh_{k}", tag="hh"
                    )
                half = hh_pair[:, :, (k % 2) * TOK : (k % 2 + 1) * TOK]
                nc.vector.tensor_mul(
                    half,
                    sg[:].rearrange("p (c n) -> p c n", c=NCH),
                    ups[:].rearrange("p (c n) -> p c n", c=NCH),
                )

                if k % 2 == 1:
                    pend.append((hh_pair, h, t))
            # epilogue: two pairs remain; overlay into the two most recent
            # gate-psum slots (one per parity -> distinct banks, no serialize)
            emit_down_pair(pend.pop(0), gps_hist[K - 2][:, : 2 * TOK])
            emit_down_pair(pend.pop(0), gps_hist[K - 1][:, : 2 * TOK])
    nc.compile()
    return nc


def _shard_inputs(inputs):
    import ml_dtypes

    bf16 = ml_dtypes.bfloat16
    x = np.asarray(inputs["x"], dtype=np.float32)
    gw = np.asarray(inputs["gate_w"], dtype=np.float32).astype(bf16)
    uw = np.asarray(inputs["up_w"], dtype=np.float32).astype(bf16)
    dw = np.asarray(inputs["down_w"], dtype=np.float32).astype(bf16)

    xh = x.reshape(T, HEADS, HD)
    xt = np.ascontiguousarray(xh.transpose(1, 2, 0)).astype(bf16)  # [16, 128, T]

    in_maps = []
    for c in range(N_CORES):
        hs = slice(HPC * c, HPC * (c + 1))
        in_maps.append(
            {
                "xT": xt[hs],
                "gw": gw[hs],
                "uw": uw[hs],
                "dw": dw[hs],
            }
        )
    return in_maps


def run(inputs, trace=False, **spmd_kwargs):
    nc = _build_nc()
    in_maps = _shard_inputs(inputs)
    res = run_bass_kernel_spmd(
        nc, in_maps, core_ids=list(range(N_CORES)), trace=trace, **spmd_kwargs
    )
    outT = np.empty((HEADS, HD, T), dtype=np.float32)
    for c in range(N_CORES):
        outT[HPC * c : HPC * (c + 1)] = np.asarray(
            res.results[c]["outT"], dtype=np.float32
        )
    out = np.ascontiguousarray(outT.transpose(2, 0, 1)).reshape(B, S, EMB)
    return out, res


def kernel(**inputs):
    out, _ = run(inputs)
    return out
